# revision 2
# baseline (speedup 1.0000x reference)
"""Trainium2 Bass kernel for nn_KeyDecider: per-(b,ch) spatial softmax +
soft-argmax + confidence, batch-sharded across 8 NeuronCores.

Input : x [64, 34, 256, 256] f32
Output: [64, 17, 3] f32  (co_x, co_y, confidence)

Math (per b, c<17):  w = softmax(x[b,c].ravel());  v = x[b,c+17].ravel()
  ki = round(sum(w*p));  out = [ki%256, ki//256, sum(w*v)]
exp() needs no max-subtraction here (inputs are randn, |x|<6), so a single
pass over HBM suffices.  The device computes, per partition-segment and
2048-wide chunk: sum(e), sum(e*t_local), sum(e*v).  The host combines the
partials in float64, folding in the (segment_offset * sum(e)) term exactly.
"""

import sys

for _p in ("/opt/trn_rl_repo", "/root/.axon_site/_ro/trn_rl_repo"):
    if _p not in sys.path:
        sys.path.insert(0, _p)

import numpy as np

B, C, K, N = 64, 34, 17, 256 * 256
W = H = 256
IMG_W = IMG_H = 256.0
NCORES = 8
BPC = B // NCORES          # batches per core
SEG = 16                   # segments per spatial row; 8*16 = 128 partitions
SEGLEN = N // SEG          # 4096
FT = 2048                  # chunk width (free dim per instruction)
NT = SEGLEN // FT          # chunks per segment
COLS = K * NT              # stats columns per core

_cache = {}


def _build(reps: int = 1):
    import concourse.bass as bass
    import concourse.bacc as bacc
    import concourse.tile as tile
    from concourse import mybir

    f32 = mybir.dt.float32
    nc = bacc.Bacc("TRN2", target_bir_lowering=False, debug=False)
    x_d = nc.declare_dram_parameter("x", [BPC, C, N], f32, isOutput=False)
    s0_d = nc.declare_dram_parameter("s0", [128, COLS], f32, isOutput=True)
    s1_d = nc.declare_dram_parameter("s1", [128, COLS], f32, isOutput=True)
    s2_d = nc.declare_dram_parameter("s2", [128, COLS], f32, isOutput=True)
    x_ap = x_d[:]

    with tile.TileContext(nc) as tc:
        with (
            tc.tile_pool(name="hp", bufs=3) as hp,
            tc.tile_pool(name="vp", bufs=3) as vp,
            tc.tile_pool(name="ep", bufs=3) as ep,
            tc.tile_pool(name="s1p", bufs=3) as s1p,
            tc.tile_pool(name="s2p", bufs=3) as s2p,
            tc.tile_pool(name="const", bufs=1) as const,
            tc.tile_pool(name="stats", bufs=1) as stats,
        ):
            pb_i = const.tile([128, FT], mybir.dt.int32)
            nc.gpsimd.iota(pb_i[:], pattern=[[1, FT]], base=0, channel_multiplier=0)
            pb = const.tile([128, FT], f32)
            nc.vector.tensor_copy(pb[:], pb_i[:])

            s0_t = stats.tile([128, COLS], f32)
            s1_t = stats.tile([128, COLS], f32)
            s2_t = stats.tile([128, COLS], f32)

            for _ in range(reps):
                for c in range(K):
                    for t in range(NT):
                        col = c * NT + t
                        # src: [b(8) x s(16)] partitions, FT contiguous elems
                        src_h = bass.AP(
                            tensor=x_ap.tensor,
                            offset=c * N + t * FT,
                            ap=[[C * N, BPC], [SEGLEN, SEG], [1, FT]],
                        )
                        src_v = bass.AP(
                            tensor=x_ap.tensor,
                            offset=(K + c) * N + t * FT,
                            ap=[[C * N, BPC], [SEGLEN, SEG], [1, FT]],
                        )
                        ht = hp.tile([128, FT], f32)
                        nc.sync.dma_start(out=ht[:], in_=src_h)
                        vt = vp.tile([128, FT], f32)
                        nc.sync.dma_start(out=vt[:], in_=src_v)

                        # ACT: e = exp(h), s0 partial fused
                        et = ep.tile([128, FT], f32)
                        nc.scalar.activation(
                            et[:], ht[:], mybir.ActivationFunctionType.Exp,
                            accum_out=s0_t[:, col:col + 1],
                        )
                        # DVE mul, ACT in-place copy w/ accum: s1 partial
                        sc1 = s1p.tile([128, FT], f32)
                        nc.vector.tensor_tensor(
                            out=sc1[:], in0=et[:], in1=pb[:],
                            op=mybir.AluOpType.mult,
                        )
                        nc.scalar.activation(
                            sc1[:], sc1[:], mybir.ActivationFunctionType.Identity,
                            accum_out=s1_t[:, col:col + 1],
                        )
                        # GPSIMD mul, DVE reduce: s2 partial
                        sc2 = s2p.tile([128, FT], f32)
                        nc.gpsimd.tensor_mul(sc2[:], et[:], vt[:])
                        nc.vector.reduce_sum(
                            s2_t[:, col:col + 1], sc2[:],
                            axis=mybir.AxisListType.X,
                        )

            nc.sync.dma_start(out=s0_d[:], in_=s0_t[:])
            nc.sync.dma_start(out=s1_d[:], in_=s1_t[:])
            nc.sync.dma_start(out=s2_d[:], in_=s2_t[:])

    nc.compile()
    return nc


def _run_device(x: np.ndarray, reps: int = 1):
    """Run the device part; returns per-core stats arrays (list of dicts)."""
    from concourse.bass_utils import run_bass_kernel_spmd

    key = reps
    if key not in _cache:
        _cache[key] = _build(reps)
    nc = _cache[key]
    in_maps = [
        {"x": np.ascontiguousarray(x[i * BPC:(i + 1) * BPC]).reshape(BPC, C, N)}
        for i in range(NCORES)
    ]
    return run_bass_kernel_spmd(nc, in_maps, list(range(NCORES)))


def _finish(results) -> np.ndarray:
    """Combine per-core partials (f64) into the [64,17,3] output."""
    out = np.empty((B, K, 3), np.float32)
    # offs[s, t] = global position of local index 0 in (segment s, chunk t)
    offs = (np.arange(SEG)[:, None] * SEGLEN
            + np.arange(NT)[None, :] * FT).astype(np.float64)  # [16, NT]
    for i in range(NCORES):
        r = results[i]
        # [128, COLS] -> [b(8), s(16), c(17), t(NT)]
        S0 = r["s0"].astype(np.float64).reshape(BPC, SEG, K, NT)
        S1 = r["s1"].astype(np.float64).reshape(BPC, SEG, K, NT)
        S2 = r["s2"].astype(np.float64).reshape(BPC, SEG, K, NT)
        o = offs[None, :, None, :]
        s0 = S0.sum(axis=(1, 3))                       # [8, 17]
        s1 = (S1 + o * S0).sum(axis=(1, 3))
        s2 = S2.sum(axis=(1, 3))
        ki = np.round(s1 / s0)
        co_x = np.mod(ki, W) / W * IMG_W
        co_y = np.floor(ki / W) / H * IMG_H
        vi = s2 / s0
        out[i * BPC:(i + 1) * BPC] = np.stack(
            [co_x, co_y, vi], axis=-1).astype(np.float32)
    return out


def kernel(x: np.ndarray) -> np.ndarray:
    res = _run_device(x, reps=1)
    return _finish(res.results)
